# revision 1
# baseline (speedup 1.0000x reference)
"""Multi-head attention block on 8 Trainium2 NeuronCores.

Reference computation (per batch b of 4, N=2048, D=768, 12 heads x 64):
    qkv = x @ Wqkv; q,k,v = split(qkv)
    out = softmax(q @ k.T / 8) @ v   (per head)
    y   = concat_heads(out) @ Wout + bout

Sharding: 8 cores = 4 batches x 2 head-groups (6 heads each).  Each core
computes its batch's QKV projection for its 6 heads, full attention for
those heads, and a partial output projection (contracting only its heads'
rows of Wout).  The host sums the two head-group partials per batch and
adds the bias (the "all-reduce after to_out" done on host).

On-core dataflow (all fp32 data; matmul APs optionally bitcast to
float32r, which streams at full PE rate instead of fp32's 4 cycles/row):
  - qT,kT computed directly transposed [d, n] (W stationary, xT moving)
  - v computed in natural [n, d] layout (xT tiles stationary, Wv moving)
    with a constant 1.0 column appended per head
  - scores computed transposed sT[n_k, n_q] = kT_tile.T @ qT, two heads
    packed in the PE array via row tiling (K=64 each)
  - exp on ScalarE (scale=1/8 fused), PSUM -> SBUF
  - out^T[d, n_q] += v_aug.T @ pT accumulated over n_k tiles; row 64 of
    the augmented output is the softmax denominator (free)
  - normalize: DVE reciprocal of denom row, K=1 matmul broadcasts it
    across partitions, DVE multiply
  - output projection contracts the head dim (outT is already the
    required lhsT layout)
"""

import os
import sys
import numpy as np

for _p in ("/opt/trn_rl_repo", "/opt/pypackages"):
    if os.path.isdir(_p) and _p not in sys.path:
        sys.path.append(_p)

import concourse.bass as bass
import concourse.mybir as mybir
import concourse.tile as tile
from concourse import bacc

# NOTE: the old f32r baseline patched --enable-ldw-opt=true (walrus
# optimizes the implicit weight loads of self-loading f32r matmuls).
# bf16 matmuls emit explicit InstLdweights, which that pass rejects,
# so the patch is disabled for the bf16 kernel.
LDW_OPT = True
import concourse.bass_utils as _bass_utils
if not getattr(_bass_utils, "_ldw_opt_patched", False):
    _orig_run_command = _bass_utils.run_command

    def _run_command_ldw(cmd, **kw):
        if LDW_OPT:
            cmd = ["--enable-ldw-opt=true" if c == "--enable-ldw-opt=false"
                   else c for c in cmd]
        return _orig_run_command(cmd, **kw)

    _bass_utils.run_command = _run_command_ldw
    _bass_utils._ldw_opt_patched = True

F32 = mybir.dt.float32
BF16 = mybir.dt.bfloat16
# Matmul compute dtype: float32 (exact, 4 cycles/row) or float32r
# (TF32-like single pass, 1 cycle/row for moving dim >= 256).
# NOTE: fp8 attention weights (DoubleRow) were measured at 3-4% absmax
# output error -- over the 2e-2 gate -- so attention stays in f32r.
MM_DT = mybir.dt.float32r
# Stationary-operand dtypes.  f32r LDWEIGHTS is suspected to cost 4
# cycles/col on HW (unmodeled by CoreSim); bf16 stationaries load 4x
# faster.  Moving operands stay f32r except where the tile itself went
# bf16.
ST_X = BF16    # xt: stationary in v_proj, moving in qk_proj
ST_W = BF16    # wq/wk/wv weight tiles (stationary in qk_proj)
ST_K = BF16    # kT (stationary in scores)
ST_V = BF16    # v (stationary in attn@v)
ST_O = BF16    # oT (stationary in outproj)
ST_WO = BF16   # wo (moving in outproj, bf16 halves its SBUF/DMA)

P = 128          # partitions
N = 2048         # sequence length
D = 768          # model dim
HD = 64          # head dim
NHPC = 6         # heads per core
NPAIR = 3        # head pairs per core
KT = D // P      # 6 feature tiles
NT = N // P      # 16 sequence tiles
GCOLS = NHPC * HD          # 384 = this core's slice of inner dim
NQB = 2                    # n_q blocks
QB = N // NQB              # 1024 block width
EXP = mybir.ActivationFunctionType.Exp
SCALE = 1.0 / np.sqrt(HD)


def _mm(nc, out, lhsT, rhs, **kw):
    nc.tensor.matmul(out, lhsT, rhs, **kw)


def build_nc(reps=1):
    nc = bacc.Bacc("TRN2", target_bir_lowering=False, debug=False, num_devices=8)
    xT_d = nc.dram_tensor("xT", [D, N], MM_DT, kind="ExternalInput").ap()
    wq_d = nc.dram_tensor("wq", [D, GCOLS], MM_DT, kind="ExternalInput").ap()
    wk_d = nc.dram_tensor("wk", [D, GCOLS], MM_DT, kind="ExternalInput").ap()
    wv_d = nc.dram_tensor("wv", [D, GCOLS], MM_DT, kind="ExternalInput").ap()
    wo_d = nc.dram_tensor("wo", [GCOLS, D], MM_DT, kind="ExternalInput").ap()
    out_d = nc.dram_tensor("out", [N, D], F32, kind="ExternalOutput").ap()

    with tile.TileContext(nc) as tc, \
         nc.allow_low_precision(reason="float32r matmul inputs"):
      for _rep in range(reps):
        with tc.tile_pool(name="persist", bufs=1) as pp:
            ones = pp.tile([1, HD], MM_DT, tag="ones")
            nc.vector.memset(ones[:].bitcast(F32), 1.0)
            qT = pp.tile([P, NPAIR, N], MM_DT, tag="qT")
            kT = pp.tile([P, NPAIR, N], MM_DT, tag="kT")
            v = pp.tile([P, NT, NHPC, HD + 1], MM_DT, tag="v")
            oT = pp.tile([P, NPAIR, N], MM_DT, tag="oT")
            wo_sb = pp.tile([P, NPAIR, D], MM_DT, tag="wo")
            nc.vector.memset(v[:, :, :, HD:HD + 1].bitcast(F32), 1.0)
            wo_dma_started = []

            def start_wo_dma():
                if not wo_dma_started:
                    for hp in range(NPAIR):
                        nc.gpsimd.dma_start(wo_sb[:, hp, :],
                                            wo_d[hp * P:(hp + 1) * P, :])
                    wo_dma_started.append(True)

            # ---- QKV + attention, interleaved per head-pair ----
            with tc.tile_pool(name="stage1", bufs=1) as wp, \
                 tc.tile_pool(name="psP", bufs=2, space="PSUM") as psP, \
                 tc.tile_pool(name="ptp", bufs=4) as ptp, \
                 tc.tile_pool(name="rp", bufs=1) as rp:
                psA = psS = psO = psP
                xt = wp.tile([P, KT, N], MM_DT, tag="xt")
                wv_sb = wp.tile([P, KT, GCOLS], MM_DT, tag="wv")
                wq0 = wp.tile([P, KT, P], MM_DT, tag="wq", bufs=2)
                wk0 = wp.tile([P, KT, P], MM_DT, tag="wk", bufs=2)
                # Startup DMA split across both hwdge queues: SP streams the
                # first-needed xT halves (cc0), the Activation queue carries
                # wq0/wk0 (tiny, needed first), wv, then the xT cc1 halves.
                # Attention can start once the cc0 halves + wq0/wk0 land.
                for kt in range(KT):
                    rows = slice(kt * P, (kt + 1) * P)
                    nc.scalar.dma_start(wq0[:, kt, :], wq_d[rows, 0:P])
                    nc.scalar.dma_start(wk0[:, kt, :], wk_d[rows, 0:P])
                    nc.sync.dma_start(xt[:, kt, 0:QB], xT_d[rows, 0:QB])
                for kt in range(KT):
                    rows = slice(kt * P, (kt + 1) * P)
                    nc.gpsimd.dma_start(wv_sb[:, kt, :], wv_d[rows, :])
                for kt in range(KT):
                    rows = slice(kt * P, (kt + 1) * P)
                    nc.gpsimd.dma_start(xt[:, kt, QB:N], xT_d[rows, QB:N])
                # wo queues behind wv and xT cc1 on the Pool queue (it is
                # not needed until the first outproj, much later)
                start_wo_dma()

                def v_proj(nts):
                    # xT tile stationary, Wv moving -> natural [n, d]
                    for nt in nts:
                        psv = psP.tile([P, GCOLS], F32, tag="s", name="psv")
                        for kt in range(KT):
                            _mm(nc, psv[:],
                                xt[:, kt, nt * P:(nt + 1) * P],
                                wv_sb[:, kt, :],
                                start=(kt == 0), stop=(kt == KT - 1))
                        nc.vector.tensor_copy(
                            v[:, nt, :, 0:HD],
                            psv[:].rearrange("p (h d) -> p h d", h=NHPC))

                obp = ptp  # output tiles share the pt slots

                def outproj(nts, queues=None):
                    qs = queues or (nc.sync, nc.gpsimd)
                    for j, nt in enumerate(nts):
                        ob = obp.tile([P, D], F32, tag="pt", name="ob")
                        for h in range(2):
                            hs = slice(h * 384, (h + 1) * 384)
                            po = psP.tile([P, 384], F32, tag="s", name="po")
                            for hp in range(NPAIR):
                                _mm(nc, po[:],
                                    oT[:, hp, nt * P:(nt + 1) * P],
                                    wo_sb[:, hp, hs],
                                    start=(hp == 0), stop=(hp == NPAIR - 1))
                            nc.vector.tensor_copy(ob[:, hs], po[:])
                        qs[j % len(qs)].dma_start(
                            out_d[nt * P:(nt + 1) * P, :], ob[:, 0:D])

                def qk_proj(hp, groups=None):
                    cols = slice(hp * P, (hp + 1) * P)
                    if hp == 0:
                        wq_sb, wk_sb = wq0, wk0
                    elif hp in wqk_cache:
                        wq_sb, wk_sb = wqk_cache[hp]
                    else:
                        wq_sb = wp.tile([P, KT, P], MM_DT, tag="wq", bufs=2)
                        wk_sb = wp.tile([P, KT, P], MM_DT, tag="wk", bufs=2)
                        for kt in range(KT):
                            rows = slice(kt * P, (kt + 1) * P)
                            nc.sync.dma_start(wq_sb[:, kt, :],
                                              wq_d[rows, cols])
                            nc.sync.dma_start(wk_sb[:, kt, :],
                                              wk_d[rows, cols])
                        wqk_cache[hp] = (wq_sb, wk_sb)
                    if groups is None:
                        groups = [(cc, p) for cc in range(2)
                                  for p in ((wq_sb, qT), (wk_sb, kT))]
                    else:
                        groups = [(cc, ((wq_sb, qT), (wk_sb, kT))[pi])
                                  for cc, pi in groups]
                    for cc, (wsb, dst) in groups:
                        # 512-wide half-inserts: each borrows the shared
                        # "s" psum slot for ~2us only, which the 2-deep exp
                        # buffer hides (a full 1024-wide insert stalls ACT)
                        for ci in range(2):
                            c = cc * 2 + ci
                            ps2 = psP.tile([P, 512], F32, tag="s", name="ps2")
                            for kt in range(KT):
                                _mm(nc, ps2[:],
                                    wsb[:, kt, :],
                                    xt[:, kt, c * 512:(c + 1) * 512],
                                    start=(kt == 0), stop=(kt == KT - 1))
                            nc.vector.tensor_copy(
                                dst[:, hp, c * 512:(c + 1) * 512], ps2[:])

                def attn_block(hp, b2, mids=None, act_norm_copy=False):
                    # Dual-head blocks: two exp chains (h2=0,1), one sc slot
                    # each.  Measured fastest on HW (447us); a single-head
                    # depth-2 variant and a bf16 variant both measured
                    # slower (487/476us) -- the HW attention loop appears
                    # floored by ScalarE-PSUM contention, not chain depth.
                    if True:
                        oacc = [psO.tile([HD + 1, QB], F32, tag="o",
                                         name=f"oacc{h2}")
                                for h2 in range(2)]
                        for i in range(NT):
                            sc = [psS.tile([P, QB], F32, tag="s",
                                           name=f"sc{h2}")
                                  for h2 in range(2)]
                            kslc = slice(i * P, (i + 1) * P)
                            for h2, lo in ((0, 0), (1, HD)):
                                for c in range(2):
                                    qs = slice(b2 * QB + c * 512,
                                               b2 * QB + (c + 1) * 512)
                                    _mm(nc, sc[h2][:, c * 512:(c + 1) * 512],
                                        kT[lo:lo + HD, hp, kslc],
                                        qT[lo:lo + HD, hp, qs],
                                        tile_position=(lo, 0))
                            pt = [ptp.tile([P, QB], MM_DT, tag="pt",
                                           name=f"pt{h2}")
                                  for h2 in range(2)]
                            for h2 in range(2):
                                nc.scalar.activation(
                                    pt[h2][:], sc[h2][:], EXP, scale=SCALE)
                            for h2 in range(2):
                                for c in range(2):
                                    cs = slice(c * 512, (c + 1) * 512)
                                    _mm(nc, oacc[h2][:, cs],
                                        v[:, i, 2 * hp + h2, :],
                                        pt[h2][:, cs],
                                        start=(i == 0), stop=(i == NT - 1))
                            if mids and i in mids:
                                mids[i]()
                        # evacuate fast (frees psum); normalize off-path.
                        # bcp reuses oacc's own rows 0:HD (already copied
                        # out) -- no extra psum slot, no "o"-tag pressure.
                        nqs = slice(b2 * QB, (b2 + 1) * QB)
                        for h2 in range(2):
                            oslc = oT[h2 * HD:(h2 + 1) * HD, hp, nqs]
                            r = rp.tile([1, QB], MM_DT, tag="r")
                            if act_norm_copy:
                                nc.scalar.copy(oslc, oacc[h2][0:HD, :])
                            else:
                                nc.vector.tensor_copy(oslc, oacc[h2][0:HD, :])
                            nc.vector.reciprocal(r[:], oacc[h2][HD:HD + 1, :])
                            bcp = oacc[h2][0:HD, :]
                            for c in range(2):
                                cs = slice(c * 512, (c + 1) * 512)
                                _mm(nc, bcp[:, cs], ones[:, :], r[:, cs])
                            nc.vector.tensor_mul(oslc, oslc, bcp)

                wqk_cache = {}
                # Prologue work hides behind the ~10us xT cc0 DMA stream:
                # cc0 halves of q,k plus the first 7 v tiles (all cc0-gated).
                # cc1 halves arrive on the Pool queue and are consumed by
                # mids (k cc1 first used at i=8, q cc1 at the b2=1 blocks).
                qk_proj(0, groups=[(0, 0), (0, 1)])
                v_proj(range(3))
                attn_block(0, 0, mids={
                    1: lambda: v_proj([3, 4]),
                    2: lambda: v_proj([5]),
                    3: lambda: (v_proj([6]), qk_proj(0, groups=[(1, 1)])),
                    4: lambda: v_proj([7]),
                    5: lambda: v_proj([8]),
                    6: lambda: v_proj([9]),
                    7: lambda: v_proj([10]),
                    8: lambda: v_proj([11]),
                    9: lambda: v_proj([12]),
                    10: lambda: v_proj([13]),
                    11: lambda: (v_proj([14]), qk_proj(1, groups=[(0, 0)])),
                    12: lambda: v_proj([15]),
                    13: lambda: qk_proj(1, groups=[(0, 1)])})
                attn_block(1, 0, mids={
                    1: lambda: qk_proj(1, groups=[(1, 1)]),
                    3: lambda: qk_proj(1, groups=[(1, 0)]),
                    7: lambda: qk_proj(2, groups=[(0, 0)]),
                    9: lambda: qk_proj(2, groups=[(0, 1)])})
                attn_block(2, 0, mids={
                    1: lambda: qk_proj(2, groups=[(1, 1)]),
                    3: lambda: qk_proj(2, groups=[(1, 0)]),
                    7: lambda: qk_proj(0, groups=[(1, 0)])})
                attn_block(0, 1, mids={
                    1: lambda: outproj(range(0, 2)),
                    4: lambda: outproj(range(2, 4)),
                    7: lambda: outproj(range(4, 6)),
                    10: lambda: outproj(range(6, 8))})
                attn_block(1, 1)
                attn_block(2, 1, act_norm_copy=True)
                outproj(range(8, NT),
                        queues=(nc.sync, nc.gpsimd, nc.scalar))


    nc.compile()
    return nc


_NC_CACHE = None


def _get_nc():
    global _NC_CACHE
    if _NC_CACHE is None:
        _NC_CACHE = build_nc()
    return _NC_CACHE


def make_in_maps(x, Wqkv, Wout):
    in_maps = []
    for core in range(8):
        b, g = divmod(core, 2)
        cols = slice(g * GCOLS, (g + 1) * GCOLS)
        in_maps.append({
            "xT": np.ascontiguousarray(x[b].T),
            "wq": np.ascontiguousarray(Wqkv[:, cols]),
            "wk": np.ascontiguousarray(
                Wqkv[:, D + g * GCOLS:D + (g + 1) * GCOLS]),
            "wv": np.ascontiguousarray(
                Wqkv[:, 2 * D + g * GCOLS:2 * D + (g + 1) * GCOLS]),
            "wo": np.ascontiguousarray(
                Wout[g * GCOLS:(g + 1) * GCOLS, :]),
        })
    return in_maps


def assemble(results, bout):
    out = np.empty((4, N, D), np.float32)
    for b in range(4):
        out[b] = results[2 * b]["out"] + results[2 * b + 1]["out"] + bout[None, :]
    return out


def kernel(x, Wqkv, Wout, bout, _trace=False):
    from concourse.bass_utils import run_bass_kernel_spmd
    x = np.asarray(x, np.float32)
    Wqkv = np.asarray(Wqkv, np.float32)
    Wout = np.asarray(Wout, np.float32)
    bout = np.asarray(bout, np.float32)
    nc = _get_nc()
    res = run_bass_kernel_spmd(nc, make_in_maps(x, Wqkv, Wout),
                               list(range(8)), trace=_trace)
    out = assemble(res.results, bout)
    if _trace:
        return out, res
    return out



# revision 2
# speedup vs baseline: 1.4378x; 1.4378x over previous
"""Multi-head attention block on 8 Trainium2 NeuronCores.

Reference computation (per batch b of 4, N=2048, D=768, 12 heads x 64):
    qkv = x @ Wqkv; q,k,v = split(qkv)
    out = softmax(q @ k.T / 8) @ v   (per head)
    y   = concat_heads(out) @ Wout + bout

Sharding: 8 cores = 4 batches x 2 head-groups (6 heads each).  Each core
computes its batch's QKV projection for its 6 heads, full attention for
those heads, and a partial output projection (contracting only its heads'
rows of Wout).  The host sums the two head-group partials per batch and
adds the bias (the "all-reduce after to_out" done on host).

V3 design (ScalarE-dense pipeline).  The kernel-wide floor is the
softmax exp: 6 heads x 2048^2 elements = 25.2M / (128 lanes @ 1.2GHz)
= 164us of ScalarE busy time, plus ~0.3us per ACTIVATE.  Everything else
is scheduled around keeping ACT 100% busy:

  - all matmul tiles are bf16 (DMA'd as bf16 from host-converted
    inputs): halves DMA + SBUF, makes LDWEIGHTS cheap/FWL-able.  PSUM
    accumulation stays fp32; rel err ~1e-3, gate is 2e-2.
  - attention tick = (head-pair, 512-wide q block, k tile i): both
    heads' transposed scores in ONE psum tile sc[128, 2, 512] (2 banks)
    written by 2 row-tiled MMs; ONE 1024-wide exp (1147ns) -> pt bf16;
    attn@V one tick later (software pipeline) so PE never waits on ACT.
  - PSUM: "s" 2x2 banks (sc), "o" 2x1 (oacc, 65 rows: row 64 is the
    softmax denominator from the all-ones V column), "p" 2x1 dedicated
    to the projection GEMMs -- proj work never steals the exp chain's
    banks (the previous kernel's main stall).
  - x is DMA'd column-chunk-major so the first q/k chunks + v tiles are
    ready ~4us in; qkv/out projections are emitted between ticks (mids)
    sized ~1us so the 2-deep sc/pt buffers hide them.
"""

import os
import sys
import numpy as np

for _p in ("/opt/trn_rl_repo", "/opt/pypackages"):
    if os.path.isdir(_p) and _p not in sys.path:
        sys.path.append(_p)

import concourse.bass as bass
import concourse.mybir as mybir
import concourse.tile as tile
from concourse import bacc

# bf16 matmuls emit explicit InstLdweights, which walrus's ldw-opt pass
# (only relevant for self-loading f32r matmuls) crashes on -- keep the
# default --enable-ldw-opt=false.  Microbench showed f32r/bf16 weight
# loads are hidden by the PE reorder window anyway (same-stationary vs
# new-stationary-every-MM differ by ~12ns/MM).
LDW_OPT = False
import concourse.bass_utils as _bass_utils
if not getattr(_bass_utils, "_ldw_opt_patched", False):
    _orig_run_command = _bass_utils.run_command

    def _run_command_ldw(cmd, **kw):
        if LDW_OPT:
            cmd = ["--enable-ldw-opt=true" if c == "--enable-ldw-opt=false"
                   else c for c in cmd]
        return _orig_run_command(cmd, **kw)

    _bass_utils.run_command = _run_command_ldw
    _bass_utils._ldw_opt_patched = True

F32 = mybir.dt.float32
F32R = mybir.dt.float32r
BF16 = mybir.dt.bfloat16

P = 128          # partitions
N = 2048         # sequence length
D = 768          # model dim
HD = 64          # head dim
NHPC = 6         # heads per core
NPAIR = 3        # head pairs per core
KT = D // P      # 6 feature tiles
NT = N // P      # 16 sequence tiles
GCOLS = NHPC * HD          # 384 = this core's slice of inner dim
NQB = 4                    # n_q blocks
QB = N // NQB              # 512 block width
EXP = mybir.ActivationFunctionType.Exp
SCALE = 1.0 / np.sqrt(HD)


def build_nc(reps=1):
    nc = bacc.Bacc("TRN2", target_bir_lowering=False, debug=False,
                   num_devices=8)
    xT_d = nc.dram_tensor("xT", [D, N], BF16, kind="ExternalInput").ap()
    wq_d = nc.dram_tensor("wq", [D, GCOLS], BF16, kind="ExternalInput").ap()
    wk_d = nc.dram_tensor("wk", [D, GCOLS], BF16, kind="ExternalInput").ap()
    wv_d = nc.dram_tensor("wv", [D, GCOLS], BF16, kind="ExternalInput").ap()
    wo_d = nc.dram_tensor("wo", [GCOLS, D], BF16, kind="ExternalInput").ap()
    out_d = nc.dram_tensor("out", [N, D], F32, kind="ExternalOutput").ap()

    with tile.TileContext(nc) as tc, \
         nc.allow_low_precision(reason="bf16 matmuls"):
      for _rep in range(reps):
        with tc.tile_pool(name="persist", bufs=1) as pp, \
             tc.tile_pool(name="psS", bufs=2, space="PSUM") as psS, \
             tc.tile_pool(name="psO", bufs=2, space="PSUM") as psO, \
             tc.tile_pool(name="psP", bufs=2, space="PSUM") as psP, \
             tc.tile_pool(name="ptp", bufs=3) as ptp, \
             tc.tile_pool(name="obp", bufs=3) as obp, \
             tc.tile_pool(name="rp", bufs=2) as rp:
            ones = pp.tile([1, HD], F32R, tag="ones")
            nc.vector.memset(ones[:].bitcast(F32), 1.0)
            qT = pp.tile([P, NPAIR, N], BF16, tag="qT")
            kT = pp.tile([P, NPAIR, N], BF16, tag="kT")
            v = pp.tile([P, NT, NHPC, HD + 1], BF16, tag="v")
            oT = pp.tile([P, NPAIR, N], BF16, tag="oT")
            wo_sb = pp.tile([P, NPAIR, D], BF16, tag="wo")
            xt = pp.tile([P, KT, N], BF16, tag="xt")
            wv_sb = pp.tile([P, KT, GCOLS], BF16, tag="wv")
            wq_sb = pp.tile([P, NPAIR, KT, P], BF16, tag="wqs")
            wk_sb = pp.tile([P, NPAIR, KT, P], BF16, tag="wks")
            nc.vector.memset(v[:, :, :, HD:HD + 1], 1.0)

            # ---- input DMA schedule ----
            # ACT HWDGE ring: head-pair-0 q/k weights (needed first),
            # then the rest of the q/k weights.
            for kt in range(KT):
                rows = slice(kt * P, (kt + 1) * P)
                nc.scalar.dma_start(wq_sb[:, 0, kt, :], wq_d[rows, 0:P])
                nc.scalar.dma_start(wk_sb[:, 0, kt, :], wk_d[rows, 0:P])
            # SP HWDGE ring: x, column-chunk-major so chunk c lands
            # ~2.5us after chunk c-1.
            for c in range(NQB):
                cols = slice(c * QB, (c + 1) * QB)
                for kt in range(KT):
                    rows = slice(kt * P, (kt + 1) * P)
                    nc.sync.dma_start(xt[:, kt, cols], xT_d[rows, cols])
            # SWDGE: wv (needed by v_proj(0) right away), then wo.
            for kt in range(KT):
                rows = slice(kt * P, (kt + 1) * P)
                nc.gpsimd.dma_start(wv_sb[:, kt, :], wv_d[rows, :])
            for hp in range(1, NPAIR):
                cols = slice(hp * P, (hp + 1) * P)
                for kt in range(KT):
                    rows = slice(kt * P, (kt + 1) * P)
                    nc.scalar.dma_start(wq_sb[:, hp, kt, :],
                                        wq_d[rows, cols])
                    nc.scalar.dma_start(wk_sb[:, hp, kt, :],
                                        wk_d[rows, cols])
            for hp in range(NPAIR):
                nc.gpsimd.dma_start(wo_sb[:, hp, :],
                                    wo_d[hp * P:(hp + 1) * P, :])

            # ---- projection helpers (dedicated "p" psum tag) ----
            def qk_proj(hp, which, c):
                w_sb, dst = ((wq_sb, qT) if which == "q" else (wk_sb, kT))
                cols = slice(c * QB, (c + 1) * QB)
                ps = psP.tile([P, QB], F32, tag="p", name="ps")
                for kt in range(KT):
                    nc.tensor.matmul(ps[:], w_sb[:, hp, kt, :],
                                     xt[:, kt, cols],
                                     start=(kt == 0), stop=(kt == KT - 1))
                nc.vector.tensor_copy(dst[:, hp, cols], ps[:])

            def v_proj(nt):
                psv = psP.tile([P, QB], F32, tag="p", name="psv")
                for kt in range(KT):
                    nc.tensor.matmul(psv[:, 0:GCOLS],
                                     xt[:, kt, nt * P:(nt + 1) * P],
                                     wv_sb[:, kt, :],
                                     start=(kt == 0), stop=(kt == KT - 1))
                nc.vector.tensor_copy(
                    v[:, nt, :, 0:HD],
                    psv[:, 0:GCOLS].rearrange("p (h d) -> p h d", h=NHPC))

            out_q = [nc.sync, nc.gpsimd]

            def outproj(nt):
                ob = obp.tile([P, D], F32, tag="ob", name="ob")
                for h in range(2):
                    hs = slice(h * GCOLS, (h + 1) * GCOLS)
                    po = psP.tile([P, QB], F32, tag="p", name="po")
                    for hp in range(NPAIR):
                        nc.tensor.matmul(po[:, 0:GCOLS],
                                         oT[:, hp, nt * P:(nt + 1) * P],
                                         wo_sb[:, hp, hs],
                                         start=(hp == 0),
                                         stop=(hp == NPAIR - 1))
                    nc.vector.tensor_copy(ob[:, hs], po[:, 0:GCOLS])
                out_q[nt % 2].dma_start(out_d[nt * P:(nt + 1) * P, :],
                                        ob[:, 0:D])

            def do_unit(u):
                if u[0] == "v":
                    v_proj(u[1])
                elif u[0] == "qk":
                    qk_proj(u[1], u[2], u[3])
                else:
                    outproj(u[1])

            # ---- attention block: 16 ticks, av one tick behind ----
            def attn_block(hp, b2, mids):
                nqs = slice(b2 * QB, (b2 + 1) * QB)
                oacc = [psO.tile([HD + 1, QB], F32, tag="o",
                                 name=f"oacc{h2}") for h2 in range(2)]
                prev_pt = None
                for i in range(NT + 1):
                    pt = None
                    if i < NT:
                        sc = psS.tile([P, 2, QB], F32, tag="s", name="sc")
                        kslc = slice(i * P, (i + 1) * P)
                        for h2, lo in ((0, 0), (1, HD)):
                            nc.tensor.matmul(
                                sc[:, h2, :],
                                kT[lo:lo + HD, hp, kslc],
                                qT[lo:lo + HD, hp, nqs],
                                start=True, stop=True,
                                tile_position=(lo, 0))
                        pt = ptp.tile([P, 2, QB], BF16, tag="pt", name="pt")
                        nc.scalar.activation(pt[:], sc[:], EXP, scale=SCALE)
                    if i >= 1:
                        for h2 in range(2):
                            nc.tensor.matmul(
                                oacc[h2][:], v[:, i - 1, 2 * hp + h2, :],
                                prev_pt[:, h2, :],
                                start=(i == 1), stop=(i == NT))
                    prev_pt = pt
                    for u in mids.get(i, ()):
                        do_unit(u)
                # evacuate + normalize (row HD of oacc is the denom)
                for h2 in range(2):
                    oslc = oT[h2 * HD:(h2 + 1) * HD, hp, nqs]
                    nc.vector.tensor_copy(oslc, oacc[h2][0:HD, :])
                    r = rp.tile([1, QB], F32R, tag="r")
                    nc.vector.reciprocal(r[:], oacc[h2][HD:HD + 1, :])
                    bcp = oacc[h2][0:HD, :]
                    nc.tensor.matmul(bcp[:, :], ones[:, :], r[:, :],
                                     start=True, stop=True)
                    nc.vector.tensor_mul(oslc, oslc, bcp)

            # ---- prologue compute ----
            qk_proj(0, "q", 0)
            qk_proj(0, "k", 0)
            for nt in range(3):
                v_proj(nt)

            # ---- static schedule ----
            SCHED = {
                (0, 0): {0: [("qk", 0, "k", 1), ("v", 3)],
                         1: [("v", 4)], 2: [("v", 5)], 3: [("v", 6)],
                         4: [("qk", 0, "k", 2)],
                         5: [("v", 7)], 6: [("v", 8)], 7: [("v", 9)],
                         8: [("qk", 0, "k", 3)],
                         9: [("v", 10)], 10: [("v", 11)], 11: [("v", 12)],
                         12: [("v", 13)], 13: [("v", 14)],
                         14: [("v", 15), ("qk", 0, "q", 1)]},
                (0, 1): {1: [("qk", 1, "k", 0)], 3: [("qk", 1, "k", 1)],
                         5: [("qk", 1, "k", 2)], 7: [("qk", 1, "k", 3)],
                         9: [("qk", 1, "q", 0)], 11: [("qk", 0, "q", 2)]},
                (0, 2): {1: [("qk", 0, "q", 3)], 5: [("qk", 1, "q", 1)]},
                (0, 3): {1: [("qk", 1, "q", 2)]},
                (1, 0): {1: [("qk", 2, "k", 0)], 3: [("qk", 2, "k", 1)],
                         5: [("qk", 2, "k", 2)], 7: [("qk", 2, "k", 3)],
                         9: [("qk", 2, "q", 0)], 11: [("qk", 1, "q", 3)]},
                (1, 1): {1: [("qk", 2, "q", 1)]},
                (1, 2): {1: [("qk", 2, "q", 2)]},
                (1, 3): {1: [("qk", 2, "q", 3)]},
                (2, 0): {},
                (2, 1): {1: [("out", 0)], 5: [("out", 1)],
                         9: [("out", 2)], 13: [("out", 3)]},
                (2, 2): {1: [("out", 4)], 5: [("out", 5)],
                         9: [("out", 6)], 13: [("out", 7)]},
                (2, 3): {1: [("out", 8)], 5: [("out", 9)],
                         9: [("out", 10)], 13: [("out", 11)]},
            }
            for hp in range(NPAIR):
                for b2 in range(NQB):
                    attn_block(hp, b2, SCHED[(hp, b2)])
            for nt in range(12, NT):
                outproj(nt)

    nc.compile()
    return nc


_NC_CACHE = None


def _get_nc():
    global _NC_CACHE
    if _NC_CACHE is None:
        _NC_CACHE = build_nc()
    return _NC_CACHE


def make_in_maps(x, Wqkv, Wout):
    import ml_dtypes
    bf16 = ml_dtypes.bfloat16
    in_maps = []
    for core in range(8):
        b, g = divmod(core, 2)
        cols = slice(g * GCOLS, (g + 1) * GCOLS)
        in_maps.append({
            "xT": np.ascontiguousarray(x[b].T).astype(bf16),
            "wq": np.ascontiguousarray(Wqkv[:, cols]).astype(bf16),
            "wk": np.ascontiguousarray(
                Wqkv[:, D + g * GCOLS:D + (g + 1) * GCOLS]).astype(bf16),
            "wv": np.ascontiguousarray(
                Wqkv[:, 2 * D + g * GCOLS:2 * D + (g + 1) * GCOLS]
            ).astype(bf16),
            "wo": np.ascontiguousarray(
                Wout[g * GCOLS:(g + 1) * GCOLS, :]).astype(bf16),
        })
    return in_maps


def assemble(results, bout):
    out = np.empty((4, N, D), np.float32)
    for b in range(4):
        out[b] = results[2 * b]["out"] + results[2 * b + 1]["out"] + bout[None, :]
    return out


def kernel(x, Wqkv, Wout, bout, _trace=False):
    from concourse.bass_utils import run_bass_kernel_spmd
    x = np.asarray(x, np.float32)
    Wqkv = np.asarray(Wqkv, np.float32)
    Wout = np.asarray(Wout, np.float32)
    bout = np.asarray(bout, np.float32)
    nc = _get_nc()
    res = run_bass_kernel_spmd(nc, make_in_maps(x, Wqkv, Wout),
                               list(range(8)), trace=_trace)
    out = assemble(res.results, bout)
    if _trace:
        return out, res
    return out
